# revision 1
# baseline (speedup 1.0000x reference)
"""Trainium2 Bass kernel for the ModelB graph loss.

Strategy (data-parallel over batch, 8 batches per core):
  * node_masks are contiguous prefix masks; each batch's valid region is the
    top-left [n, n] block of its [N, N] matrices.  The host extracts that
    block (gathering first if a mask is ever non-prefix - the sums are node-
    permutation invariant) and packs it, padded, into per-(core, slot) blocks
    whose shapes are shared by all 8 cores, so one SPMD program serves all
    cores.  Batches with n <= 50 are packed two per block at partition
    offsets 0/64; their accumulator columns are split by partition range on
    the host.
  * Pad fills are P=0.5, A=0, R=0: every reduced quantity then has a pad
    contribution of exactly zero except sum(ln(1-P)), sum(ln(P)) and
    sum(P^2).  The ln pads are pad_count*ln(.5) (the device's own value is
    read back through a calibration accumulator lane) and cancel exactly in
    sum(DLT) = sum(LP) - sum(L1P); the P^2 pad is pad_count*0.25.
  * On device each block is a [128, T*N] bf16 SBUF tile (DRAM image is
    pre-interleaved so every partition's span is contiguous).  Per-batch
    masked sums come out as per-partition row-sum columns via fused
    accum_out on ACT/DVE ops; products with no accumulator needs run on
    GPSIMD.  ARI-branch quantities are only computed for slots holding
    n <= 50 batches.  The host reduces the [128, x] stats tensors and
    finishes the scalar arithmetic in float64.
"""

import sys

for _p in ("/opt/trn_rl_repo", "/root/.axon_site/_ro/trn_rl_repo"):
    if _p not in sys.path:
        sys.path.insert(0, _p)

from contextlib import ExitStack

import numpy as np

import concourse.bass as bass  # noqa: F401  (registers engine methods)
import concourse.tile as tile
from concourse import bacc, mybir
from concourse.bass_utils import run_bass_kernel_spmd

N_CORES = 8
B, N, C = 64, 512, 2
N_SLOTS = B // N_CORES  # 8 batches per core
EPS = 1e-8

# stats_v per-slot cols: [AD, SD2, PD, PA, P2, DLT] (large: AD/SD2/DLT)
QV = 6
# stats_a per-slot columns: [LP, L1P, SA, ABS, SD2a] (SD2a: big slots'
# SD^2 row-sum accumulated via ACT Square to offload DVE)
QA = 5

_FT = mybir.dt.float32
_BF = mybir.dt.bfloat16
_AF = mybir.ActivationFunctionType
_OP = mybir.AluOpType

try:
    import ml_dtypes

    _BF_NP = ml_dtypes.bfloat16
except ImportError:  # pragma: no cover
    _BF_NP = None

_build_cache: dict = {}


def _plan(n_list):
    """Choose slot shapes shared by all cores and assign batches to them.

    Rank batches by n descending; rank-group s (8 consecutive ranks) gives
    one batch to every core.  Groups whose members all fit in 64 partitions
    are merged pairwise into "P" slots holding two batches per core at
    partition offsets 0/64.

    Returns (sig, assign) where sig is a tuple of slot descriptors
    ("F", ns, ts, ari) or ("P", f, ari), and assign maps
    (core, slot_index, sub) -> batch index.
    """
    n_arr = np.asarray(n_list)
    order = np.argsort(-n_arr, kind="stable")
    groups = []
    for s in range(N_SLOTS):
        g = order[s * N_CORES : (s + 1) * N_CORES]
        groups.append((int(max(n_arr[b] for b in g)), [int(b) for b in g]))

    sig = []
    assign = {}
    slot = 0
    s = 0
    while s < N_SLOTS:
        ns, g = groups[s]
        if s + 1 < N_SLOTS and ns <= 64 and groups[s + 1][0] <= 64:
            ns2, g2 = groups[s + 1]
            f = max(ns, ns2)
            ari = bool(any(n_arr[b] <= 50 for b in g + g2))
            sig.append(("P", f, ari))
            for c in range(N_CORES):
                assign[(c, slot, 0)] = g[c]
                assign[(c, slot, 1)] = g2[c]
            s += 2
        else:
            ts = max(1, -(-ns // 128))
            ari = bool(any(n_arr[b] <= 50 for b in g))
            # split tall blocks into row-range sub-slots of <=2 segments
            # (finer pipeline grain; all sums split cleanly across rows)
            row0 = 0
            while row0 < ts:
                tseg = ts - row0
                sig.append(("F", ns, tseg, ari, row0))
                for c in range(N_CORES):
                    assign[(c, slot, 0)] = g[c]
                slot += 1
                row0 += tseg
            s += 1
            continue
        slot += 1
    return tuple(sig), assign


def _slot_f(e):
    return e[1] * e[2] if e[0] == "F" else e[1]


def _build(sig):
    nc = bacc.Bacc("TRN2", target_bir_lowering=False, debug=False,
                   num_devices=N_CORES)

    p_in, a_in, r_in = [], [], []
    for s, e in enumerate(sig):
        f = _slot_f(e)
        p_in.append(nc.dram_tensor(f"p{s}", [128, f], _BF,
                                   kind="ExternalInput").ap())
        a_in.append(nc.dram_tensor(f"a{s}", [128, f], _BF,
                                   kind="ExternalInput").ap())
        r_in.append(nc.dram_tensor(f"r{s}", [128, f], _BF,
                                   kind="ExternalInput").ap())
    pc_in = nc.dram_tensor("pc", [128, 64], _FT, kind="ExternalInput").ap()
    pt_in = nc.dram_tensor("pt", [128, 64], _FT, kind="ExternalInput").ap()
    mc_in = nc.dram_tensor("mc", [128, 64], _FT, kind="ExternalInput").ap()
    cal_in = nc.dram_tensor("cal", [1, 2], _BF, kind="ExternalInput").ap()
    nslots = len(sig)
    sv_cols = nslots * QV + 2
    sa_cols = nslots * QA + 1
    sv_out = nc.dram_tensor("sv", [128, sv_cols], _FT,
                            kind="ExternalOutput").ap()
    sa_out = nc.dram_tensor("sa", [128, sa_cols], _FT,
                            kind="ExternalOutput").ap()

    with tile.TileContext(nc) as tc, ExitStack() as ctx:
        pp = ctx.enter_context(tc.tile_pool(name="pp", bufs=4))
        pa = ctx.enter_context(tc.tile_pool(name="pa", bufs=4))
        pr = ctx.enter_context(tc.tile_pool(name="pr", bufs=4))
        pmid = ctx.enter_context(tc.tile_pool(name="pmid", bufs=5))
        pdum = ctx.enter_context(tc.tile_pool(name="pdum", bufs=6))
        pstat = ctx.enter_context(tc.tile_pool(name="pstat", bufs=1))
        psml = ctx.enter_context(tc.tile_pool(name="psml", bufs=1))

        stats_v = pstat.tile([128, sv_cols], _FT, tag="sv")
        stats_a = pstat.tile([128, sa_cols], _FT, tag="sa")
        bm05 = pstat.tile([128, 1], _FT, tag="bm05")
        nc.gpsimd.memset(bm05[:], -0.5)
        bm1 = pstat.tile([128, 1], _FT, tag="bm1")
        nc.gpsimd.memset(bm1[:], -1.0)

        def svc(s, q):
            col = s * QV + q
            return stats_v[:, col : col + 1]

        def sac(s, q):
            col = s * QA + q
            return stats_a[:, col : col + 1]

        # coordinate inputs early on the gpsimd issuer (sync carries the
        # big F-slot loads; compute for these goes mid-stream)
        tpc = psml.tile([128, 64], _FT, tag="tpc")
        nc.gpsimd.dma_start(tpc[:], pc_in[:])
        tpt = psml.tile([128, 64], _FT, tag="tpt")
        nc.gpsimd.dma_start(tpt[:], pt_in[:])
        tmc = psml.tile([128, 64], _FT, tag="tmc")
        nc.gpsimd.dma_start(tmc[:], mc_in[:])
        tcal = psml.tile([1, 2], _BF, tag="tcal")
        nc.gpsimd.dma_start(tcal[:], cal_in[:])

        fslots = [s for s, e in enumerate(sig) if e[0] == "F"]
        pslots = [s for s, e in enumerate(sig) if e[0] == "P"]
        build_order = fslots[:1] + pslots + fslots[1:]
        # dma_starts are completion-serialized on their issuing engine
        # (~600ns+ each); sync carries the large F-slot loads in slot
        # order, gpsimd carries pair-slot and coordinate loads
        def chunked_load(tile_, src_, f, eng):
            eng.dma_start(tile_[:], src_[:])

        mid_emit = (pslots[-1] if pslots else build_order[0])

        def emit_coords():
            # coordinate losses, packed [128, 64] over all 8 local batches
            d = psml.tile([128, 64], _FT, tag="d")
            nc.vector.tensor_sub(d[:], tpc[:], tpt[:])
            dm = psml.tile([128, 64], _FT, tag="dm")
            nc.vector.tensor_mul(dm[:], d[:], tmc[:])
            dsml = psml.tile([128, 64], _FT, tag="dsml")
            nc.vector.scalar_tensor_tensor(
                dsml[:], dm[:], 1.0, dm[:], _OP.mult, _OP.mult,
                accum_out=stats_v[:, nslots * QV : nslots * QV + 1])
            adm = psml.tile([128, 64], _FT, tag="adm")
            nc.scalar.activation(adm[:], dm[:], _AF.Abs)
            hb = psml.tile([128, 64], _FT, tag="hb")
            nc.scalar.activation(hb[:], adm[:], _AF.Relu, bias=bm1[:])
            dsml2 = psml.tile([128, 64], _FT, tag="dsml2")
            nc.vector.scalar_tensor_tensor(
                dsml2[:], hb[:], 1.0, hb[:], _OP.mult, _OP.mult,
                accum_out=stats_v[:, nslots * QV + 1 : nslots * QV + 2])

            # calibration: mirror the L1P op on pad-valued input; the fp32
            # accumulator then reports exactly 2x the per-element pad term.
            dcal = psml.tile([1, 2], _BF, tag="dcal")
            nc.scalar.activation(dcal[:], tcal[:], _AF.Ln, bias=1.0, scale=-1.0,
                                 accum_out=stats_a[0:1, nslots * QA :
                                                   nslots * QA + 1])



        for s in build_order:
            e = sig[s]
            f = _slot_f(e)
            ari = e[-1]
            ldeng = nc.sync if e[0] == "F" else nc.gpsimd
            tp = pp.tile([128, f], _BF, tag="tp")
            chunked_load(tp, p_in[s], f, ldeng)
            ta = pa.tile([128, f], _BF, tag="ta")
            chunked_load(ta, a_in[s], f, ldeng)
            tr = pr.tile([128, f], _BF, tag="tr")
            chunked_load(tr, r_in[s], f, ldeng)

            # ACT: the two logs; their accums give S_LP, S_L1P (and so
            # S_DLT = S_LP - S_L1P on the host, ln(.5) pads cancelling)
            lp = pmid.tile([128, f], _BF, tag="lp")
            nc.scalar.activation(lp[:], tp[:], _AF.Ln,
                                 accum_out=sac(s, 0))
            l1p = pmid.tile([128, f], _BF, tag="l1p")
            nc.scalar.activation(l1p[:], tp[:], _AF.Ln, bias=1.0, scale=-1.0,
                                 accum_out=sac(s, 1))

            # similarity difference: V tensor_tensor is 2x for bf16; use
            # GPSIMD only for smaller slots to keep V for the big ones
            sd = pmid.tile([128, f], _BF, tag="sd")
            if f >= 1500:
                nc.vector.tensor_sub(sd[:], tr[:], ta[:])
            else:
                nc.gpsimd.tensor_sub(sd[:], tr[:], ta[:])

            dlt = pmid.tile([128, f], _BF, tag="dlt")
            nc.vector.tensor_sub(dlt[:], lp[:], l1p[:])

            # DVE: products whose row sums we need
            dv = pdum.tile([128, f], _BF, tag="dv")
            nc.vector.scalar_tensor_tensor(
                dv[:], ta[:], 1.0, dlt[:], _OP.mult, _OP.mult,
                accum_out=svc(s, 0))
            if f >= 1500:
                da4 = pdum.tile([128, f], _BF, tag="da")
                nc.scalar.activation(da4[:], sd[:], _AF.Square,
                                     accum_out=sac(s, 4))
            else:
                dv = pdum.tile([128, f], _BF, tag="dv")
                nc.vector.scalar_tensor_tensor(
                    dv[:], sd[:], 1.0, sd[:], _OP.mult, _OP.mult,
                    accum_out=svc(s, 1))

            if ari:
                # quantities consumed only by the n <= 50 ARI branch
                da2 = pdum.tile([128, f], _BF, tag="da")
                nc.scalar.activation(da2[:], ta[:], _AF.Copy,
                                     accum_out=sac(s, 2))
                da3 = pdum.tile([128, f], _BF, tag="da")
                nc.scalar.activation(da3[:], tp[:], _AF.Abs, bias=bm05[:],
                                     accum_out=sac(s, 3))
                dv = pdum.tile([128, f], _BF, tag="dv")
                nc.vector.scalar_tensor_tensor(
                    dv[:], tp[:], 1.0, dlt[:], _OP.mult, _OP.mult,
                    accum_out=svc(s, 2))
                dv = pdum.tile([128, f], _BF, tag="dv")
                nc.vector.scalar_tensor_tensor(
                    dv[:], tp[:], 1.0, ta[:], _OP.mult, _OP.mult,
                    accum_out=svc(s, 3))
                dv = pdum.tile([128, f], _BF, tag="dv")
                nc.vector.scalar_tensor_tensor(
                    dv[:], tp[:], 1.0, tp[:], _OP.mult, _OP.mult,
                    accum_out=svc(s, 4))

            if s == mid_emit:
                emit_coords()

        nc.sync.dma_start(sv_out[:], stats_v[:])
        nc.sync.dma_start(sa_out[:], stats_a[:])

    nc.compile()
    return nc


def _huber(x):
    ax = np.abs(x)
    return np.where(ax <= 1.0, 0.5 * x * x, ax - 0.5)


def _interleave(block, ts):
    """[ts*128, n] row-major -> [128, ts*n] with per-partition contiguity."""
    if ts == 1:
        return block
    n = block.shape[1]
    return np.ascontiguousarray(
        block.reshape(ts, 128, n).transpose(1, 0, 2).reshape(128, ts * n))


def kernel(predicted_coords, adjacency_matrix, node_counts, raw_similarity,
           temperature, residual_weight, points, adjacency, node_masks,
           _want_results=None):
    masks = np.asarray(node_masks).astype(bool)
    n_list = masks.sum(axis=1).astype(np.int64)
    sig, assign = _plan(n_list)

    if sig not in _build_cache:
        _build_cache[sig] = _build(sig)
    nc = _build_cache[sig]

    p_full = np.asarray(adjacency_matrix, dtype=np.float32)
    a_full = np.asarray(adjacency, dtype=np.float32)
    r_full = np.asarray(raw_similarity, dtype=np.float32)
    pc_full = np.ascontiguousarray(predicted_coords, dtype=np.float32)
    pt_full = np.ascontiguousarray(points, dtype=np.float32)
    m_f32 = masks.astype(np.float32)

    # valid-node index per batch (prefix fast path; gather fallback)
    valid = []
    for b in range(B):
        n = int(n_list[b])
        if masks[b, :n].all():
            valid.append(None)  # prefix: plain slicing
        else:
            valid.append(np.flatnonzero(masks[b]))

    in_maps = []
    for c in range(N_CORES):
        im = {}
        bs = []
        for s, e in enumerate(sig):
            if e[0] == "F":
                _, ns, ts, ari, row0 = e
                b = assign[(c, s, 0)]
                n = int(n_list[b])
                r0 = row0 * 128
                r1 = min(n, r0 + ts * 128)
                nr = max(0, r1 - r0)
                bp = np.full((ts * 128, ns), 0.5, np.float32)
                ba = np.zeros((ts * 128, ns), np.float32)
                br = np.zeros((ts * 128, ns), np.float32)
                if nr > 0:
                    if valid[b] is None:
                        bp[:nr, :n] = p_full[b, r0:r1, :n]
                        ba[:nr, :n] = a_full[b, r0:r1, :n]
                        br[:nr, :n] = r_full[b, r0:r1, :n]
                    else:
                        ix = np.ix_(valid[b][r0:r1], valid[b])
                        bp[:nr, :n] = p_full[b][ix]
                        ba[:nr, :n] = a_full[b][ix]
                        br[:nr, :n] = r_full[b][ix]
                im[f"p{s}"] = _interleave(bp, ts).astype(_BF_NP)
                im[f"a{s}"] = _interleave(ba, ts).astype(_BF_NP)
                im[f"r{s}"] = _interleave(br, ts).astype(_BF_NP)
                if row0 == 0:
                    bs.append(b)
            else:
                _, f, ari = e
                bp = np.full((128, f), 0.5, np.float32)
                ba = np.zeros((128, f), np.float32)
                br = np.zeros((128, f), np.float32)
                for sub, off in ((0, 0), (1, 64)):
                    b = assign[(c, s, sub)]
                    n = int(n_list[b])
                    if valid[b] is None:
                        bp[off:off + n, :n] = p_full[b, :n, :n]
                        ba[off:off + n, :n] = a_full[b, :n, :n]
                        br[off:off + n, :n] = r_full[b, :n, :n]
                    else:
                        ix = np.ix_(valid[b], valid[b])
                        bp[off:off + n, :n] = p_full[b][ix]
                        ba[off:off + n, :n] = a_full[b][ix]
                        br[off:off + n, :n] = r_full[b][ix]
                    bs.append(b)
                im[f"p{s}"] = bp.astype(_BF_NP)
                im[f"a{s}"] = ba.astype(_BF_NP)
                im[f"r{s}"] = br.astype(_BF_NP)
        im["pc"] = pc_full[bs].reshape(128, 64)
        im["pt"] = pt_full[bs].reshape(128, 64)
        im["mc"] = np.repeat(m_f32[bs][:, :, None], C, axis=2).reshape(128, 64)
        im["cal"] = np.array([[0.5, 0.5]], _BF_NP)
        in_maps.append(im)

    res = run_bass_kernel_spmd(nc, in_maps, core_ids=list(range(N_CORES)))
    if _want_results is not None:
        _want_results.append(res)

    # ---- host finalization in float64 ----
    nslots = len(sig)
    sv = [res.results[c]["sv"].astype(np.float64) for c in range(N_CORES)]
    sa = [res.results[c]["sa"].astype(np.float64) for c in range(N_CORES)]
    lnhalf = float(res.results[0]["sa"][0, nslots * QA]) / 2.0

    n_arr = n_list.astype(np.float64)
    cnt_coord = max(float(n_arr.sum()) * C, 1.0)
    cnt2 = max(float((n_arr ** 2).sum()), 1.0)

    s_mse = sum(float(v[:, nslots * QV].sum()) for v in sv)
    s_hsq = sum(float(v[:, nslots * QV + 1].sum()) for v in sv)
    coord_mse = s_mse / cnt_coord
    coord_smooth = (0.5 * s_mse - 0.5 * s_hsq) / cnt_coord
    coord_loss = 0.7 * coord_mse + 0.3 * coord_smooth

    edge_sum = 0.0
    sim_sum = 0.0
    # per-batch accumulation first (a batch may span several row-split
    # slots); the nonlinear ARI math runs once per batch afterwards
    acc = {b: [0.0] * 7 for b in range(B)}  # L1P, DLT, AD, SD2, PD, PA, P2
    acc_a = {b: [0.0] * 2 for b in range(B)}  # SA, ABS
    for c in range(N_CORES):
        for s, e in enumerate(sig):
            if e[0] == "F":
                _, ns, ts, ari, row0 = e
                subs = [(assign[(c, s, 0)], 0, 128, float(ts * 128 * ns),
                         row0, ts)]
            else:
                _, f, ari = e
                subs = [(assign[(c, s, 0)], 0, 64, float(64 * f), 0, 0),
                        (assign[(c, s, 1)], 64, 128, float(64 * f), 0, 0)]
            for b, lo, hi, area, row0, ts_ in subs:
                n = float(n_list[b])
                if e[0] == "F":
                    r0 = row0 * 128
                    nr = max(0.0, min(n, r0 + ts_ * 128) - r0)
                    real = nr * n
                else:
                    real = n * n
                padcnt = area - real
                s_lp = float(sa[c][lo:hi, s * QA + 0].sum())
                s_l1p_raw = float(sa[c][lo:hi, s * QA + 1].sum())
                a = acc[b]
                a[0] += s_l1p_raw - padcnt * lnhalf
                a[1] += s_lp - s_l1p_raw  # ln(.5) pads cancel exactly
                a[2] += float(sv[c][lo:hi, s * QV + 0].sum())
                if e[0] == "F" and _slot_f(e) >= 1500:
                    a[3] += float(sa[c][lo:hi, s * QA + 4].sum())
                else:
                    a[3] += float(sv[c][lo:hi, s * QV + 1].sum())
                if ari:
                    a[4] += float(sv[c][lo:hi, s * QV + 2].sum())
                    a[5] += float(sv[c][lo:hi, s * QV + 3].sum())
                    a[6] += float(sv[c][lo:hi, s * QV + 4].sum()) \
                        - padcnt * 0.25
                    aa = acc_a[b]
                    aa[0] += float(sa[c][lo:hi, s * QA + 2].sum())
                    aa[1] += float(sa[c][lo:hi, s * QA + 3].sum())

    ari_loss = 0.0
    conf_pen = 0.0
    for b in range(B):
        n = float(n_list[b])
        s_l1p, s_dlt, s_ad, s_sd2, s_pd, s_pa, s_p2 = acc[b]
        edge_sum += s_l1p + 0.05 * s_dlt + 0.9 * s_ad
        sim_sum += s_sd2
        if 5.0 < n <= 50.0:
            s_a, s_abs = acc_a[b]
            na = np.sqrt(max(s_p2, 0.0))
            nt = np.sqrt(max(s_a, 0.0))
            cos = s_pa / (max(na, EPS) * max(nt, EPS))
            n2 = max(n * n, 1.0)
            ent = -(s_l1p + s_pd) / n2
            contrast = s_abs / n2
            ari_loss += -cos - 0.2 * contrast
            conf_pen += ent

    edge_loss = -edge_sum / cnt2
    similarity_loss = sim_sum / cnt2

    dc = np.asarray(node_counts, np.float64) - n_arr
    count_loss = float(_huber(dc).mean())
    temp_reg = abs(float(temperature) - 1.0)
    res_reg = abs(float(residual_weight) - 0.5)

    total = (1.0 * coord_loss + 2.0 * edge_loss + 0.1 * count_loss
             + 0.3 * similarity_loss + 0.01 * (temp_reg + res_reg)
             + 1.0 * (ari_loss + 0.1 * conf_pen))
    return np.asarray(total, dtype=np.float32)



# revision 11
# speedup vs baseline: 1.2827x; 1.2827x over previous
"""Trainium2 Bass kernel for the ModelB graph loss.

Strategy (data-parallel over batch, 8 batches per core, dense packing):
  * node_masks are contiguous prefix masks; each batch's valid region is the
    top-left [n, n] block of its [N, N] matrices.  The host flattens that
    block (gathering first if a mask is ever non-prefix) and packs it
    row-major into whole partition-rows of a single [128, F1] stream per
    tensor, one stream per core.  Batches are assigned to cores by greedy
    LPT on n^2 so all cores carry ~equal element counts; F1 is the smallest
    width that fits the worst core in 128 partitions.
  * Per-batch masked sums come out as fused fp32 accum_out columns
    (per-partition row sums); the host adds each batch's row range.
    Pad fills are P=0.5, A=0, R=0: every reduced quantity then has a pad
    contribution of exactly zero except sum(ln p) / sum(ln(1-p)) (pad
    ln(0.5) read back via a calibration lane; it cancels in the dlt and
    p*dlt differences), sum(p^2) (0.25/элem) and sum(max/min(p,.5))
    (cancels in the max-min difference).
  * ACT does only the two big Ln passes (+2 tiny aux Ln, cal); DVE does the
    products/sums; |x| needs come from max/min/abs_max ALU ops so no extra
    ACT passes.  Small batches (n<=50) additionally appear in a [128, F2]
    aux stream for the ARI-only quantities.  The host finishes the scalar
    arithmetic in float64.
"""

import sys

for _p in ("/opt/trn_rl_repo", "/root/.axon_site/_ro/trn_rl_repo"):
    if _p not in sys.path:
        sys.path.insert(0, _p)

from contextlib import ExitStack

import numpy as np

import concourse.bass as bass  # noqa: F401  (registers engine methods)
import concourse.tile as tile
from concourse import bacc, mybir
from concourse.bass_utils import run_bass_kernel_spmd

N_CORES = 8
B, N, C = 64, 512, 2
PER_CORE = B // N_CORES
EPS = 1e-8

_FT = mybir.dt.float32
_BF = mybir.dt.bfloat16
_AF = mybir.ActivationFunctionType
_OP = mybir.AluOpType

try:
    import ml_dtypes

    _BF_NP = ml_dtypes.bfloat16
except ImportError:  # pragma: no cover
    _BF_NP = None

_build_cache: dict = {}

# stats columns (fp32, [128, 16])
C_LP, C_L1P, C_L1PB, C_ALP, C_AL1P, C_AL1PB, C_SD2 = 0, 1, 2, 3, 4, 5, 6
C_PLP, C_PL1P, C_P2, C_PMAX, C_PMIN, C_PA, C_SA = 7, 8, 9, 10, 11, 12, 13
C_DM2, C_H2 = 14, 15
NCOL = 16

# device Ln(0.5): the ACT spline is exact to ~1e-6 here and the pad-count
# correction tolerates far more, so no calibration lane is needed.
LNHALF = float(np.log(0.5))


def _plan(n_list):
    """Greedy-LPT assign 8 batches to each core; choose stream widths.

    Returns (sig, cores) with sig=(F1, F2) and cores a list of per-core
    batch-index lists (each exactly PER_CORE long, big batches first).
    """
    n2 = np.asarray(n_list, dtype=np.int64) ** 2
    order = np.argsort(-n2, kind="stable")
    loads = [0] * N_CORES
    counts = [0] * N_CORES
    cores = [[] for _ in range(N_CORES)]
    for b in order:
        c = min(
            (c for c in range(N_CORES) if counts[c] < PER_CORE),
            key=lambda c: loads[c],
        )
        cores[c].append(int(b))
        loads[c] += int(n2[b])
        counts[c] += 1

    def fits(width, vals):
        return sum(-(-int(v) // width) for v in vals) <= 128

    F1 = max(512, (max(loads) // 126 // 16 + 1) * 16)
    while not all(fits(F1, n2[cores[c]]) for c in range(N_CORES)):
        F1 += 16
    F2 = 32
    while not all(
        fits(F2, [n2[b] for b in cores[c] if n_list[b] <= 50])
        for c in range(N_CORES)
    ):
        F2 += 32
    return (int(F1), int(F2)), cores


def _build(sig):
    F1, F2 = sig
    nc = bacc.Bacc("TRN2", target_bir_lowering=False, debug=False,
                   num_devices=N_CORES)

    p_in = nc.dram_tensor("p", [128, F1], _BF, kind="ExternalInput").ap()
    a_in = nc.dram_tensor("a", [128, F1], _BF, kind="ExternalInput").ap()
    r_in = nc.dram_tensor("r", [128, F1], _BF, kind="ExternalInput").ap()
    px_in = nc.dram_tensor("px", [128, F2], _BF, kind="ExternalInput").ap()
    ax_in = nc.dram_tensor("ax", [128, F2], _BF, kind="ExternalInput").ap()
    pc_in = nc.dram_tensor("pc", [128, 64], _FT, kind="ExternalInput").ap()
    pt_in = nc.dram_tensor("pt", [128, 64], _FT, kind="ExternalInput").ap()
    mc_in = nc.dram_tensor("mc", [128, 64], _FT, kind="ExternalInput").ap()
    st_out = nc.dram_tensor("st", [128, NCOL], _FT,
                            kind="ExternalOutput").ap()

    with tile.TileContext(nc) as tc, ExitStack() as ctx:
        po = ctx.enter_context(tc.tile_pool(name="po", bufs=1))

        st = po.tile([128, NCOL], _FT, tag="st")

        tp = po.tile([128, F1], _BF, tag="tp")
        ta = po.tile([128, F1], _BF, tag="ta")
        tr = po.tile([128, F1], _BF, tag="tr")
        lp = po.tile([128, F1], _BF, tag="lp")
        l1p = po.tile([128, F1], _BF, tag="l1p")
        sd = po.tile([128, F1], _BF, tag="sd")
        sd2o = po.tile([128, F1], _BF, tag="sd2o")
        alpo = po.tile([128, F1], _BF, tag="alpo")
        al1po = po.tile([128, F1], _BF, tag="al1po")

        tpx = po.tile([128, F2], _BF, tag="tpx")
        tax = po.tile([128, F2], _BF, tag="tax")
        lpx = po.tile([128, F2], _BF, tag="lpx")
        l1px = po.tile([128, F2], _BF, tag="l1px")
        x0 = po.tile([128, F2], _BF, tag="x0")
        x1 = po.tile([128, F2], _BF, tag="x1")
        x2 = po.tile([128, F2], _BF, tag="x2")
        x3 = po.tile([128, F2], _BF, tag="x3")
        x4 = po.tile([128, F2], _BF, tag="x4")
        x5 = po.tile([128, F2], _BF, tag="x5")
        x6 = po.tile([128, F2], _BF, tag="x6")

        tpc = po.tile([128, 64], _FT, tag="tpc")
        tpt = po.tile([128, 64], _FT, tag="tpt")
        tmc = po.tile([128, 64], _FT, tag="tmc")
        cd = po.tile([128, 64], _FT, tag="cd")
        cdm = po.tile([128, 64], _FT, tag="cdm")
        cdo = po.tile([128, 64], _FT, tag="cdo")
        cad = po.tile([128, 64], _FT, tag="cad")
        ch = po.tile([128, 64], _FT, tag="ch")
        cho = po.tile([128, 64], _FT, tag="cho")

        # ---- DMA: p/r on sync (HWDGE); a + small inputs on gpsimd (SWDGE)
        nc.sync.dma_start(tp[:], p_in[:])
        nc.sync.dma_start(tr[:], r_in[:])
        nc.gpsimd.dma_start(ta[:], a_in[:])
        nc.gpsimd.dma_start(tpx[:], px_in[:])
        nc.gpsimd.dma_start(tax[:], ax_in[:])
        nc.gpsimd.dma_start(tpc[:], pc_in[:])
        nc.gpsimd.dma_start(tpt[:], pt_in[:])
        nc.gpsimd.dma_start(tmc[:], mc_in[:])

        # ---- ACT queue; ln(1-p) in halves so the a*ln(1-p) product
        # overlaps the second half
        h1 = (F1 // 2 + 3) // 4 * 4
        nc.scalar.activation(lp[:], tp[:], _AF.Ln,
                             accum_out=st[:, C_LP:C_LP + 1])
        nc.scalar.activation(l1p[:, :h1], tp[:, :h1], _AF.Ln,
                             bias=1.0, scale=-1.0,
                             accum_out=st[:, C_L1P:C_L1P + 1])
        nc.scalar.activation(l1p[:, h1:], tp[:, h1:], _AF.Ln,
                             bias=1.0, scale=-1.0,
                             accum_out=st[:, C_L1PB:C_L1PB + 1])
        nc.scalar.activation(lpx[:], tpx[:], _AF.Ln)
        nc.scalar.activation(l1px[:], tpx[:], _AF.Ln, bias=1.0, scale=-1.0)

        # ---- DVE queue (emission order = schedule)
        # big stream: similarity first (needs only r, a)
        nc.vector.tensor_sub(sd[:], tr[:], ta[:])
        nc.vector.scalar_tensor_tensor(
            sd2o[:], sd[:], 1.0, sd[:], _OP.mult, _OP.mult,
            accum_out=st[:, C_SD2:C_SD2 + 1])
        # coordinate loss, fully on DVE
        nc.vector.tensor_sub(cd[:], tpc[:], tpt[:])
        nc.vector.tensor_mul(cdm[:], cd[:], tmc[:])
        nc.vector.scalar_tensor_tensor(
            cdo[:], cdm[:], 1.0, cdm[:], _OP.mult, _OP.mult,
            accum_out=st[:, C_DM2:C_DM2 + 1])
        # h = max(|dm| - 1, 0) with |dm| = max(dm, -dm); plain DVE ALU ops
        nc.vector.tensor_scalar_mul(cad[:], cdm[:], -1.0)
        nc.vector.tensor_tensor(ch[:], cdm[:], cad[:], _OP.max)
        nc.vector.tensor_scalar_add(cho[:], ch[:], -1.0)
        nc.vector.tensor_scalar_max(ch[:], cho[:], 0.0)
        nc.vector.scalar_tensor_tensor(
            cd[:], ch[:], 1.0, ch[:], _OP.mult, _OP.mult,
            accum_out=st[:, C_H2:C_H2 + 1])
        # aux (small-batch ARI extras) not needing the aux logs
        nc.vector.scalar_tensor_tensor(
            x0[:], tpx[:], 1.0, tpx[:], _OP.mult, _OP.mult,
            accum_out=st[:, C_P2:C_P2 + 1])
        nc.vector.tensor_scalar(x1[:], tpx[:], 0.5, None, _OP.max, _OP.add,
                                accum_out=st[:, C_PMAX:C_PMAX + 1])
        nc.vector.tensor_scalar(x2[:], tpx[:], 0.5, None, _OP.min, _OP.add,
                                accum_out=st[:, C_PMIN:C_PMIN + 1])
        nc.vector.scalar_tensor_tensor(
            x3[:], tpx[:], 1.0, tax[:], _OP.mult, _OP.mult,
            accum_out=st[:, C_PA:C_PA + 1])
        nc.vector.scalar_tensor_tensor(
            x4[:], tax[:], 1.0, tax[:], _OP.mult, _OP.mult,
            accum_out=st[:, C_SA:C_SA + 1])
        # a * log products (gated on the ACT outputs)
        nc.vector.scalar_tensor_tensor(
            alpo[:], ta[:], 1.0, lp[:], _OP.mult, _OP.mult,
            accum_out=st[:, C_ALP:C_ALP + 1])
        nc.vector.scalar_tensor_tensor(
            al1po[:, :h1], ta[:, :h1], 1.0, l1p[:, :h1], _OP.mult, _OP.mult,
            accum_out=st[:, C_AL1P:C_AL1P + 1])
        nc.vector.scalar_tensor_tensor(
            al1po[:, h1:], ta[:, h1:], 1.0, l1p[:, h1:], _OP.mult, _OP.mult,
            accum_out=st[:, C_AL1PB:C_AL1PB + 1])
        # aux products over the aux logs
        nc.vector.scalar_tensor_tensor(
            x5[:], tpx[:], 1.0, lpx[:], _OP.mult, _OP.mult,
            accum_out=st[:, C_PLP:C_PLP + 1])
        nc.vector.scalar_tensor_tensor(
            x6[:], tpx[:], 1.0, l1px[:], _OP.mult, _OP.mult,
            accum_out=st[:, C_PL1P:C_PL1P + 1])

        nc.sync.dma_start(st_out[:], st[:])

    nc.compile()
    return nc


def _huber(x):
    ax = np.abs(x)
    return np.where(ax <= 1.0, 0.5 * x * x, ax - 0.5)


def kernel(predicted_coords, adjacency_matrix, node_counts, raw_similarity,
           temperature, residual_weight, points, adjacency, node_masks,
           _want_results=None):
    masks = np.asarray(node_masks).astype(bool)
    n_list = masks.sum(axis=1).astype(np.int64)
    sig, cores = _plan(n_list)
    F1, F2 = sig

    if sig not in _build_cache:
        _build_cache[sig] = _build(sig)
    nc = _build_cache[sig]

    p_full = np.asarray(adjacency_matrix, dtype=np.float32)
    a_full = np.asarray(adjacency, dtype=np.float32)
    r_full = np.asarray(raw_similarity, dtype=np.float32)
    pc_full = np.ascontiguousarray(predicted_coords, dtype=np.float32)
    pt_full = np.ascontiguousarray(points, dtype=np.float32)
    m_f32 = masks.astype(np.float32)

    # valid-node index per batch (prefix fast path; gather fallback)
    valid = []
    for b in range(B):
        n = int(n_list[b])
        if masks[b, :n].all():
            valid.append(None)
        else:
            valid.append(np.flatnonzero(masks[b]))

    def block(full, b):
        n = int(n_list[b])
        if valid[b] is None:
            return full[b, :n, :n]
        ix = np.ix_(valid[b], valid[b])
        return full[b][ix]

    in_maps = []
    rowmap = []   # per core: batch -> (r0, r1) in main stream
    auxmap = []   # per core: batch -> (q0, q1) in aux stream
    for c in range(N_CORES):
        bs = cores[c]
        bufs = {}
        for key, full, pad in (("p", p_full, 0.5), ("a", a_full, 0.0),
                               ("r", r_full, 0.0)):
            buf = np.full(128 * F1, pad, np.float32)
            r = 0
            for b in bs:
                n = int(n_list[b])
                nn = n * n
                buf[r * F1:r * F1 + nn] = block(full, b).ravel()
                r += -(-nn // F1)
            bufs[key] = buf.reshape(128, F1).astype(_BF_NP)
        rm = {}
        r = 0
        for b in bs:
            nn = int(n_list[b]) ** 2
            rows = -(-nn // F1)
            rm[b] = (r, r + rows)
            r += rows
        rowmap.append(rm)

        am = {}
        pxb = np.full(128 * F2, 0.5, np.float32)
        axb = np.zeros(128 * F2, np.float32)
        q = 0
        for b in bs:
            n = int(n_list[b])
            if n > 50:
                continue
            nn = n * n
            pxb[q * F2:q * F2 + nn] = block(p_full, b).ravel()
            axb[q * F2:q * F2 + nn] = block(a_full, b).ravel()
            rows = -(-nn // F2)
            am[b] = (q, q + rows)
            q += rows
        auxmap.append(am)

        im = {
            "p": bufs["p"], "a": bufs["a"], "r": bufs["r"],
            "px": pxb.reshape(128, F2).astype(_BF_NP),
            "ax": axb.reshape(128, F2).astype(_BF_NP),
            "pc": pc_full[bs].reshape(128, 64),
            "pt": pt_full[bs].reshape(128, 64),
            "mc": np.repeat(m_f32[bs][:, :, None], C, axis=2).reshape(128, 64),
        }
        in_maps.append(im)

    res = run_bass_kernel_spmd(nc, in_maps, core_ids=list(range(N_CORES)))
    if _want_results is not None:
        _want_results.append(res)

    # ---- host finalization in float64 ----
    sts = [res.results[c]["st"].astype(np.float64) for c in range(N_CORES)]
    lnhalf = LNHALF

    n_arr = n_list.astype(np.float64)
    cnt_coord = max(float(n_arr.sum()) * C, 1.0)
    cnt2 = max(float((n_arr ** 2).sum()), 1.0)

    edge_sum = 0.0
    sim_sum = 0.0
    ari_loss = 0.0
    conf_pen = 0.0
    for c in range(N_CORES):
        stc = sts[c]
        for b in cores[c]:
            n = float(n_list[b])
            nn = n * n
            r0, r1 = rowmap[c][b]
            padcnt = (r1 - r0) * F1 - nn
            s_lp_raw = float(stc[r0:r1, C_LP].sum())
            s_l1p_raw = float(stc[r0:r1, C_L1P].sum()) \
                + float(stc[r0:r1, C_L1PB].sum())
            s_l1p = s_l1p_raw - padcnt * lnhalf
            s_dlt = s_lp_raw - s_l1p_raw          # ln(.5) pads cancel
            s_ad = float(stc[r0:r1, C_ALP].sum()) \
                - float(stc[r0:r1, C_AL1P].sum()) \
                - float(stc[r0:r1, C_AL1PB].sum())  # a pads are 0
            edge_sum += s_l1p + 0.05 * s_dlt + 0.9 * s_ad
            sim_sum += float(stc[r0:r1, C_SD2].sum())

            if 5.0 < n <= 50.0:
                q0, q1 = auxmap[c][b]
                s_pd = float(stc[q0:q1, C_PLP].sum()) \
                    - float(stc[q0:q1, C_PL1P].sum())  # .5*ln(.5) pads cancel
                aux_pad = (q1 - q0) * F2 - nn
                s_p2 = float(stc[q0:q1, C_P2].sum()) - 0.25 * aux_pad
                s_abs = float(stc[q0:q1, C_PMAX].sum()) \
                    - float(stc[q0:q1, C_PMIN].sum())   # .5 pads cancel
                s_pa = float(stc[q0:q1, C_PA].sum())
                s_a = float(stc[q0:q1, C_SA].sum())
                na = np.sqrt(max(s_p2, 0.0))
                nt = np.sqrt(max(s_a, 0.0))
                cos = s_pa / (max(na, EPS) * max(nt, EPS))
                n2 = max(nn, 1.0)
                ent = -(s_l1p + s_pd) / n2
                contrast = s_abs / n2
                ari_loss += -cos - 0.2 * contrast
                conf_pen += ent

    s_mse = sum(float(v[:, C_DM2].sum()) for v in sts)
    s_hsq = sum(float(v[:, C_H2].sum()) for v in sts)
    coord_mse = s_mse / cnt_coord
    coord_smooth = (0.5 * s_mse - 0.5 * s_hsq) / cnt_coord
    coord_loss = 0.7 * coord_mse + 0.3 * coord_smooth

    edge_loss = -edge_sum / cnt2
    similarity_loss = sim_sum / cnt2

    dc = np.asarray(node_counts, np.float64) - n_arr
    count_loss = float(_huber(dc).mean())
    temp_reg = abs(float(temperature) - 1.0)
    res_reg = abs(float(residual_weight) - 0.5)

    total = (1.0 * coord_loss + 2.0 * edge_loss + 0.1 * count_loss
             + 0.3 * similarity_loss + 0.01 * (temp_reg + res_reg)
             + 1.0 * (ari_loss + 0.1 * conf_pen))
    return np.asarray(total, dtype=np.float32)


# revision 12
# speedup vs baseline: 1.7166x; 1.3383x over previous
"""Trainium2 Bass kernel for the ModelB graph loss.

Strategy (data-parallel over batch, 8 batches per core, dense packing):
  * node_masks are contiguous prefix masks; each batch's valid region is the
    top-left [n, n] block of its [N, N] matrices.  The host flattens that
    block (gathering rows/cols first if a mask is ever non-prefix) and packs
    it row-major into whole partition-rows of a single [128, F1] stream per
    tensor per core.  Batches are assigned to cores by greedy LPT on n^2 so
    all cores carry ~equal element counts.
  * adjacency is binary, so every a-weighted reduction collapses to a sum
    over the a==1 positions: the host gathers p[a==1] and r[a==1] into
    small side streams (~10% density) and the device never loads the
    adjacency tensor at all.  sum((r-a)^2) = sum(r^2) - 2*sum(r[a==1]) +
    count(a==1), with the count taken on the host during the gather.
  * Per-batch masked sums come out as fused fp32 accum_out columns
    (per-partition row sums); the host adds each batch's row range.  Pad
    fills are p=0.5 (its ln(0.5) contributions either cancel in
    differences or are subtracted exactly) and r=0.
  * ACT does the two big Ln passes plus tiny gathered/aux Ln passes; DVE
    does the products and plain sums; the host finishes in float64.
"""

import sys

for _p in ("/opt/trn_rl_repo", "/root/.axon_site/_ro/trn_rl_repo"):
    if _p not in sys.path:
        sys.path.insert(0, _p)

from contextlib import ExitStack

import numpy as np

import concourse.bass as bass  # noqa: F401  (registers engine methods)
import concourse.tile as tile
from concourse import bacc, mybir
from concourse.bass_utils import run_bass_kernel_spmd

N_CORES = 8
B, N, C = 64, 512, 2
PER_CORE = B // N_CORES
EPS = 1e-8

_FT = mybir.dt.float32
_BF = mybir.dt.bfloat16
_AF = mybir.ActivationFunctionType
_OP = mybir.AluOpType

try:
    import ml_dtypes

    _BF_NP = ml_dtypes.bfloat16
except ImportError:  # pragma: no cover
    _BF_NP = None

_build_cache: dict = {}

# stats columns (fp32, [128, 16])
C_LP, C_L1P, C_LG, C_L1G, C_SG, C_SH, C_R2 = 0, 1, 2, 3, 4, 5, 6
C_PLP, C_PL1P, C_P2, C_PMAX, C_PMIN = 7, 8, 9, 10, 11
C_DM2, C_H2 = 12, 13
NCOL = 16

# device Ln(0.5): the ACT spline is exact to ~1e-6 here and the pad-count
# correction tolerates far more, so no calibration lane is needed.
LNHALF = float(np.log(0.5))


def _fits(width, vals):
    return sum(-(-int(v) // width) for v in vals) <= 128


def _plan(n_list, g_counts):
    """Greedy-LPT assign 8 batches per core; choose stream widths.

    Returns (sig, cores): sig=(F1, F2, F3); cores = per-core batch lists.
    """
    n2 = np.asarray(n_list, dtype=np.int64) ** 2
    order = np.argsort(-n2, kind="stable")
    loads = [0] * N_CORES
    counts = [0] * N_CORES
    cores = [[] for _ in range(N_CORES)]
    for b in order:
        c = min(
            (c for c in range(N_CORES) if counts[c] < PER_CORE),
            key=lambda c: loads[c],
        )
        cores[c].append(int(b))
        loads[c] += int(n2[b])
        counts[c] += 1

    F1 = max(512, (max(loads) // 126 // 16 + 1) * 16)
    while not all(_fits(F1, n2[cores[c]]) for c in range(N_CORES)):
        F1 += 16
    F2 = 32
    while not all(
        _fits(F2, [n2[b] for b in cores[c] if n_list[b] <= 50])
        for c in range(N_CORES)
    ):
        F2 += 32
    F3 = 64
    while not all(
        _fits(F3, [g_counts[b] for b in cores[c]]) for c in range(N_CORES)
    ):
        F3 += 32
    return (int(F1), int(F2), int(F3)), cores


def _build(sig):
    F1, F2, F3 = sig
    nc = bacc.Bacc("TRN2", target_bir_lowering=False, debug=False,
                   num_devices=N_CORES)

    p_in = nc.dram_tensor("p", [128, F1], _BF, kind="ExternalInput").ap()
    r_in = nc.dram_tensor("r", [128, F1], _BF, kind="ExternalInput").ap()
    g_in = nc.dram_tensor("g", [128, F3], _BF, kind="ExternalInput").ap()
    h_in = nc.dram_tensor("h", [128, F3], _BF, kind="ExternalInput").ap()
    px_in = nc.dram_tensor("px", [128, F2], _BF, kind="ExternalInput").ap()
    pc_in = nc.dram_tensor("pc", [128, 64], _FT, kind="ExternalInput").ap()
    pt_in = nc.dram_tensor("pt", [128, 64], _FT, kind="ExternalInput").ap()
    mc_in = nc.dram_tensor("mc", [128, 64], _FT, kind="ExternalInput").ap()
    st_out = nc.dram_tensor("st", [128, NCOL], _FT,
                            kind="ExternalOutput").ap()

    with tile.TileContext(nc) as tc, ExitStack() as ctx:
        po = ctx.enter_context(tc.tile_pool(name="po", bufs=1))

        st = po.tile([128, NCOL], _FT, tag="st")

        tp = po.tile([128, F1], _BF, tag="tp")
        tr = po.tile([128, F1], _BF, tag="tr")
        lp = po.tile([128, F1], _BF, tag="lp")
        l1p = po.tile([128, F1], _BF, tag="l1p")
        r2t = po.tile([128, F1], _BF, tag="r2t")
        r2s = po.tile([128, F1], _BF, tag="r2s")

        tg = po.tile([128, F3], _BF, tag="tg")
        th = po.tile([128, F3], _BF, tag="th")
        lg = po.tile([128, F3], _BF, tag="lg")
        l1g = po.tile([128, F3], _BF, tag="l1g")
        gs = po.tile([128, F3], _BF, tag="gs")
        hs = po.tile([128, F3], _BF, tag="hs")

        tpx = po.tile([128, F2], _BF, tag="tpx")
        lpx = po.tile([128, F2], _BF, tag="lpx")
        l1px = po.tile([128, F2], _BF, tag="l1px")
        x0 = po.tile([128, F2], _BF, tag="x0")
        x1 = po.tile([128, F2], _BF, tag="x1")
        x2 = po.tile([128, F2], _BF, tag="x2")
        x5 = po.tile([128, F2], _BF, tag="x5")
        x6 = po.tile([128, F2], _BF, tag="x6")

        tpc = po.tile([128, 64], _FT, tag="tpc")
        tpt = po.tile([128, 64], _FT, tag="tpt")
        tmc = po.tile([128, 64], _FT, tag="tmc")
        cd = po.tile([128, 64], _FT, tag="cd")
        cdm = po.tile([128, 64], _FT, tag="cdm")
        cdo = po.tile([128, 64], _FT, tag="cdo")
        cad = po.tile([128, 64], _FT, tag="cad")
        ch = po.tile([128, 64], _FT, tag="ch")
        cho = po.tile([128, 64], _FT, tag="cho")

        # ---- DMA: big streams on sync (HWDGE); side inputs on gpsimd
        nc.sync.dma_start(tp[:], p_in[:])
        nc.sync.dma_start(tr[:], r_in[:])
        nc.gpsimd.dma_start(tg[:], g_in[:])
        nc.gpsimd.dma_start(th[:], h_in[:])
        nc.gpsimd.dma_start(tpx[:], px_in[:])
        nc.gpsimd.dma_start(tpc[:], pc_in[:])
        nc.gpsimd.dma_start(tpt[:], pt_in[:])
        nc.gpsimd.dma_start(tmc[:], mc_in[:])

        # ---- ACT queue: tiny logs first (their inputs land early), then
        # the two big Ln passes
        nc.scalar.activation(lg[:], tg[:], _AF.Ln,
                             accum_out=st[:, C_LG:C_LG + 1])
        nc.scalar.activation(l1g[:], tg[:], _AF.Ln, bias=1.0, scale=-1.0,
                             accum_out=st[:, C_L1G:C_L1G + 1])
        nc.scalar.activation(lpx[:], tpx[:], _AF.Ln)
        nc.scalar.activation(l1px[:], tpx[:], _AF.Ln, bias=1.0, scale=-1.0)
        nc.scalar.activation(lp[:], tp[:], _AF.Ln,
                             accum_out=st[:, C_LP:C_LP + 1])
        nc.scalar.activation(l1p[:], tp[:], _AF.Ln, bias=1.0, scale=-1.0,
                             accum_out=st[:, C_L1P:C_L1P + 1])

        # ---- DVE queue (emission order = schedule)
        # coordinate loss
        nc.vector.tensor_sub(cd[:], tpc[:], tpt[:])
        nc.vector.tensor_mul(cdm[:], cd[:], tmc[:])
        nc.vector.scalar_tensor_tensor(
            cdo[:], cdm[:], 1.0, cdm[:], _OP.mult, _OP.mult,
            accum_out=st[:, C_DM2:C_DM2 + 1])
        nc.vector.tensor_scalar_mul(cad[:], cdm[:], -1.0)
        nc.vector.tensor_tensor(ch[:], cdm[:], cad[:], _OP.max)
        nc.vector.tensor_scalar_add(cho[:], ch[:], -1.0)
        nc.vector.tensor_scalar_max(ch[:], cho[:], 0.0)
        nc.vector.scalar_tensor_tensor(
            cd[:], ch[:], 1.0, ch[:], _OP.mult, _OP.mult,
            accum_out=st[:, C_H2:C_H2 + 1])
        # aux ARI extras (small batches)
        nc.vector.scalar_tensor_tensor(
            x0[:], tpx[:], 1.0, tpx[:], _OP.mult, _OP.mult,
            accum_out=st[:, C_P2:C_P2 + 1])
        nc.vector.tensor_scalar(x1[:], tpx[:], 0.5, None, _OP.max, _OP.add,
                                accum_out=st[:, C_PMAX:C_PMAX + 1])
        nc.vector.tensor_scalar(x2[:], tpx[:], 0.5, None, _OP.min, _OP.add,
                                accum_out=st[:, C_PMIN:C_PMIN + 1])
        # gathered sums: sum(p[a==1]), sum(r[a==1]) per partition
        nc.vector.tensor_scalar(gs[:], tg[:], 0.0, None, _OP.add, _OP.add,
                                accum_out=st[:, C_SG:C_SG + 1])
        nc.vector.tensor_scalar(hs[:], th[:], 0.0, None, _OP.add, _OP.add,
                                accum_out=st[:, C_SH:C_SH + 1])
        # sum(r^2): square at 2x as tensor_tensor, reduce at 4x
        nc.vector.tensor_mul(r2t[:], tr[:], tr[:])
        nc.vector.tensor_scalar(r2s[:], r2t[:], 0.0, None, _OP.add, _OP.add,
                                accum_out=st[:, C_R2:C_R2 + 1])
        # aux products over the aux logs
        nc.vector.scalar_tensor_tensor(
            x5[:], tpx[:], 1.0, lpx[:], _OP.mult, _OP.mult,
            accum_out=st[:, C_PLP:C_PLP + 1])
        nc.vector.scalar_tensor_tensor(
            x6[:], tpx[:], 1.0, l1px[:], _OP.mult, _OP.mult,
            accum_out=st[:, C_PL1P:C_PL1P + 1])

        nc.sync.dma_start(st_out[:], st[:])

    nc.compile()
    return nc


def _huber(x):
    ax = np.abs(x)
    return np.where(ax <= 1.0, 0.5 * x * x, ax - 0.5)


def kernel(predicted_coords, adjacency_matrix, node_counts, raw_similarity,
           temperature, residual_weight, points, adjacency, node_masks,
           _want_results=None):
    masks = np.asarray(node_masks).astype(bool)
    n_list = masks.sum(axis=1).astype(np.int64)

    p_full = np.asarray(adjacency_matrix, dtype=np.float32)
    a_full = np.asarray(adjacency, dtype=np.float32)
    r_full = np.asarray(raw_similarity, dtype=np.float32)
    pc_full = np.ascontiguousarray(predicted_coords, dtype=np.float32)
    pt_full = np.ascontiguousarray(points, dtype=np.float32)
    m_f32 = masks.astype(np.float32)

    # valid-node index per batch (prefix fast path; gather fallback)
    valid = []
    for b in range(B):
        n = int(n_list[b])
        if masks[b, :n].all():
            valid.append(None)
        else:
            valid.append(np.flatnonzero(masks[b]))

    def block(full, b):
        n = int(n_list[b])
        if valid[b] is None:
            return full[b, :n, :n]
        ix = np.ix_(valid[b], valid[b])
        return full[b][ix]

    # gather p/r at a==1 positions within each valid block
    g_vals, h_vals, g_counts = [], [], []
    for b in range(B):
        am = block(a_full, b).ravel()
        idx = np.flatnonzero(am != 0.0)
        g_vals.append(block(p_full, b).ravel()[idx])
        h_vals.append(block(r_full, b).ravel()[idx])
        g_counts.append(len(idx))

    sig, cores = _plan(n_list, g_counts)
    F1, F2, F3 = sig

    if sig not in _build_cache:
        _build_cache[sig] = _build(sig)
    nc = _build_cache[sig]

    in_maps = []
    rowmap = []   # per core: batch -> (r0, r1) in main stream
    auxmap = []   # per core: batch -> (q0, q1) in aux stream
    gmap = []     # per core: batch -> (u0, u1) in gathered stream
    for c in range(N_CORES):
        bs = cores[c]
        bufs = {}
        for key, full, pad in (("p", p_full, 0.5), ("r", r_full, 0.0)):
            buf = np.full(128 * F1, pad, np.float32)
            r = 0
            for b in bs:
                n = int(n_list[b])
                nn = n * n
                buf[r * F1:r * F1 + nn] = block(full, b).ravel()
                r += -(-nn // F1)
            bufs[key] = buf.reshape(128, F1).astype(_BF_NP)
        rm = {}
        r = 0
        for b in bs:
            nn = int(n_list[b]) ** 2
            rows = -(-nn // F1)
            rm[b] = (r, r + rows)
            r += rows
        rowmap.append(rm)

        gb = np.full(128 * F3, 0.5, np.float32)
        hb = np.zeros(128 * F3, np.float32)
        gm = {}
        u = 0
        for b in bs:
            cnt = g_counts[b]
            gb[u * F3:u * F3 + cnt] = g_vals[b]
            hb[u * F3:u * F3 + cnt] = h_vals[b]
            rows = -(-cnt // F3) if cnt else 0
            gm[b] = (u, u + rows)
            u += rows
        gmap.append(gm)

        am = {}
        pxb = np.full(128 * F2, 0.5, np.float32)
        q = 0
        for b in bs:
            n = int(n_list[b])
            if n > 50:
                continue
            nn = n * n
            pxb[q * F2:q * F2 + nn] = block(p_full, b).ravel()
            rows = -(-nn // F2)
            am[b] = (q, q + rows)
            q += rows
        auxmap.append(am)

        im = {
            "p": bufs["p"], "r": bufs["r"],
            "g": gb.reshape(128, F3).astype(_BF_NP),
            "h": hb.reshape(128, F3).astype(_BF_NP),
            "px": pxb.reshape(128, F2).astype(_BF_NP),
            "pc": pc_full[bs].reshape(128, 64),
            "pt": pt_full[bs].reshape(128, 64),
            "mc": np.repeat(m_f32[bs][:, :, None], C, axis=2).reshape(128, 64),
        }
        in_maps.append(im)

    res = run_bass_kernel_spmd(nc, in_maps, core_ids=list(range(N_CORES)))
    if _want_results is not None:
        _want_results.append(res)

    # ---- host finalization in float64 ----
    sts = [res.results[c]["st"].astype(np.float64) for c in range(N_CORES)]
    lnhalf = LNHALF

    n_arr = n_list.astype(np.float64)
    cnt_coord = max(float(n_arr.sum()) * C, 1.0)
    cnt2 = max(float((n_arr ** 2).sum()), 1.0)

    edge_sum = 0.0
    sim_sum = 0.0
    ari_loss = 0.0
    conf_pen = 0.0
    for c in range(N_CORES):
        stc = sts[c]
        for b in cores[c]:
            n = float(n_list[b])
            nn = n * n
            r0, r1 = rowmap[c][b]
            u0, u1 = gmap[c][b]
            cnt_a = float(g_counts[b])
            padcnt = (r1 - r0) * F1 - nn
            s_lp_raw = float(stc[r0:r1, C_LP].sum())
            s_l1p_raw = float(stc[r0:r1, C_L1P].sum())
            s_l1p = s_l1p_raw - padcnt * lnhalf
            s_dlt = s_lp_raw - s_l1p_raw          # ln(.5) pads cancel
            s_ad = float(stc[u0:u1, C_LG].sum()) \
                - float(stc[u0:u1, C_L1G].sum())   # ln(.5) pads cancel
            edge_sum += s_l1p + 0.05 * s_dlt + 0.9 * s_ad
            s_r2 = float(stc[r0:r1, C_R2].sum())
            s_ra = float(stc[u0:u1, C_SH].sum())   # r pads are 0
            sim_sum += s_r2 - 2.0 * s_ra + cnt_a

            if 5.0 < n <= 50.0:
                q0, q1 = auxmap[c][b]
                s_pd = float(stc[q0:q1, C_PLP].sum()) \
                    - float(stc[q0:q1, C_PL1P].sum())  # .5*ln(.5) pads cancel
                aux_pad = (q1 - q0) * F2 - nn
                g_pad = (u1 - u0) * F3 - cnt_a
                s_p2 = float(stc[q0:q1, C_P2].sum()) - 0.25 * aux_pad
                s_abs = float(stc[q0:q1, C_PMAX].sum()) \
                    - float(stc[q0:q1, C_PMIN].sum())   # .5 pads cancel
                s_pa = float(stc[u0:u1, C_SG].sum()) - 0.5 * g_pad
                na = np.sqrt(max(s_p2, 0.0))
                nt = np.sqrt(max(cnt_a, 0.0))
                cos = s_pa / (max(na, EPS) * max(nt, EPS))
                n2 = max(nn, 1.0)
                ent = -(s_l1p + s_pd) / n2
                contrast = s_abs / n2
                ari_loss += -cos - 0.2 * contrast
                conf_pen += ent

    s_mse = sum(float(v[:, C_DM2].sum()) for v in sts)
    s_hsq = sum(float(v[:, C_H2].sum()) for v in sts)
    coord_mse = s_mse / cnt_coord
    coord_smooth = (0.5 * s_mse - 0.5 * s_hsq) / cnt_coord
    coord_loss = 0.7 * coord_mse + 0.3 * coord_smooth

    edge_loss = -edge_sum / cnt2
    similarity_loss = sim_sum / cnt2

    dc = np.asarray(node_counts, np.float64) - n_arr
    count_loss = float(_huber(dc).mean())
    temp_reg = abs(float(temperature) - 1.0)
    res_reg = abs(float(residual_weight) - 0.5)

    total = (1.0 * coord_loss + 2.0 * edge_loss + 0.1 * count_loss
             + 0.3 * similarity_loss + 0.01 * (temp_reg + res_reg)
             + 1.0 * (ari_loss + 0.1 * conf_pen))
    return np.asarray(total, dtype=np.float32)
